# revision 7
# baseline (speedup 1.0000x reference)
"""Trainium2 Bass kernel for nn_CausalSelfAttentionSynapse (v3).

Math (per reference):
    qk = g @ W_lift.T; q,k heads of dim D=64
    lse[b,h,t] = logsumexp_{j<=t} (q_t . k_j)
    out[b,t]  = sum_h lse[b,h,t] * w[h],  w[h] = sum_g W_proj[g,h]

Sharding: 8 cores = 4 batches x 2 head-groups (8 heads each).

Per-core design (ScalarE-bound, everything else hidden under it):
  - Host pre-transposes g[b] and the head-group's W rows into e-major
    bf16 layouts (gT, wT) -> no on-device transposes at all.
  - Lift: q/k per head pair via PE matmuls (bf16 in, fp32 PSUM),
    DVE-copied to SBUF bf16; lift chunks are interleaved into the score
    stream only where PSUM ring banks are free.
  - Scores: per (head, q-tile of 128 rows) causal row block
    [0, 128*(qi+1)) matmul'd into a bank-aligned rotating PSUM ring;
    the two heads' matmuls are interleaved chunk-by-chunk so they run
    concurrently on disjoint PE row-groups (K=64 each). The causal mask
    of the diagonal block is ADDED by one extra matmul (lhsT=I,
    rhs=strictly-upper -30000).
  - Exp in place on PSUM (ScalarE); row-sums via accum_out for the big
    tiles (qi>=8) and via DVE reduce for the small ones (fewer
    ACTIVATION_READ_ACCUMULATOR instructions on the critical engine).
  - PE warm-up matmuls + dummy exp at t=0 keep HAM at 8/8 and preload
    the exp table during the input DMA.
  - Finale: lse = ln(sums) for all 128 (head, q-tile) columns at once
    via exponent/mantissa split; weighted head-sum via a replicated
    weight tile and 3 folding adds.
"""

import numpy as np
import ml_dtypes

B, T, E, H = 4, 2048, 1024, 16
D = 64
NCORES = 8
NE = 8         # 128-row chunks of e
PAIRS = 4      # head pairs per core
MASKV = -30000.0

_CACHE = {}


def _build():
    import concourse.bass as bass  # noqa: F401
    import concourse.tile as tile
    from concourse import bacc, mybir

    f32 = mybir.dt.float32
    bf16 = mybir.dt.bfloat16
    i32 = mybir.dt.int32
    EXP = mybir.ActivationFunctionType.Exp
    LN = mybir.ActivationFunctionType.Ln
    AX = mybir.AxisListType.X
    MUL = mybir.AluOpType.mult
    ADD = mybir.AluOpType.add
    SHR = mybir.AluOpType.logical_shift_right
    SUB = mybir.AluOpType.subtract
    AND = mybir.AluOpType.bitwise_and
    OR = mybir.AluOpType.bitwise_or
    LN2 = float(np.log(2.0))

    nc = bacc.Bacc("TRN2", target_bir_lowering=False, debug=False,
                   num_devices=NCORES)

    # gt rows: half*1024 + e*128 + p ; cols: t-within-half
    g_d = nc.dram_tensor("gt", [T, E], bf16, kind="ExternalInput").ap()
    # wt rows: pair*1024 + e*128 + p ; cols: f-within-pair (q0 q1 k0 k1)
    w_d = nc.dram_tensor("wt", [4 * E, 256], bf16, kind="ExternalInput").ap()
    tri_d = nc.dram_tensor("tri", [128, 128], bf16, kind="ExternalInput").ap()
    id_d = nc.dram_tensor("identb", [128, 128], bf16,
                          kind="ExternalInput").ap()
    wr_d = nc.dram_tensor("wrep", [128, 128], f32, kind="ExternalInput").ap()
    out_d = nc.dram_tensor("out_part", [128, 16], f32,
                           kind="ExternalOutput").ap()

    with tile.TileContext(nc) as tc:
        with (
            tc.tile_pool(name="consts", bufs=1) as consts,
            tc.tile_pool(name="big", bufs=1) as big,
            tc.tile_pool(name="qkp", bufs=2) as qkp,
            tc.tile_pool(name="sums", bufs=1) as sums,
            tc.tile_pool(name="fin", bufs=1) as fin,
            tc.tile_pool(name="ps", bufs=1, space="PSUM") as ps,
        ):
            # ---- constants (first: tiny, unblock warm-up) -----------------
            tri = consts.tile([128, 128], bf16, name="tri", tag="tri")
            nc.sync.dma_start(out=tri[:], in_=tri_d[:])
            identb = consts.tile([128, 128], bf16, name="identb", tag="id")
            nc.sync.dma_start(out=identb[:], in_=id_d[:])
            wrep = consts.tile([128, 128], f32, name="wrep", tag="wrep")
            nc.sync.dma_start(out=wrep[:], in_=wr_d[:])

            # ---- big SBUF layouts (host-pretransposed, bf16) --------------
            # gT col = e*2048 + t ; wT col = e*1024 + pair*256 + fw
            gT = big.tile([128, NE * T], bf16, name="gT", tag="gT")
            wT = big.tile([128, NE * E], bf16, name="wT", tag="wT")

            def dma_wt(pr):
                for e in range(NE):
                    nc.sync.dma_start(
                        out=wT[:, e * 1024 + pr * 256: e * 1024 + pr * 256 + 256],
                        in_=w_d[pr * 1024 + e * 128: pr * 1024 + e * 128 + 128, :])

            def dma_gt(tcn):
                # global t-chunk tcn in 0..3 -> (half, tc-within-half)
                half, tch = divmod(tcn, 2)
                for e in range(NE):
                    nc.sync.dma_start(
                        out=gT[:, e * 2048 + tcn * 512: e * 2048 + tcn * 512 + 512],
                        in_=g_d[half * 1024 + e * 128: half * 1024 + e * 128 + 128,
                                tch * 512: tch * 512 + 512])

            # critical-path DMAs first
            dma_wt(0)
            dma_gt(0)

            # ---- PSUM ring (8 banks of 512 fp32, bank-aligned regions) ----
            ring = ps.tile([128, 4096], f32, name="ring", tag="ring")
            ring_pos = [0]

            def ring_alloc(nbanks):
                if ring_pos[0] + nbanks > 8:
                    ring_pos[0] = 0
                off = ring_pos[0] * 512
                ring_pos[0] += nbanks
                return off

            # PE warm-up: ~3.5us of junk matmuls (from a memset tile, no DMA
            # dependency) so HAM is at 8/8 when the real lift starts; dummy
            # exp preloads the ACT table set.
            wub = fin.tile([128, 128], bf16, name="wub", tag="wub")
            nc.vector.memset(wub[:], 1.0)
            wz = fin.tile([128, 1], f32, name="wz", tag="wz")
            nc.vector.memset(wz[:], 0.0)
            joff = ring_alloc(1)
            for _ in range(128):
                nc.tensor.matmul(ring[:, joff:joff + 128], lhsT=wub[:],
                                 rhs=wub[:], start=True, stop=True)
            jx = fin.tile([128, 1], f32, name="jx", tag="jx")
            nc.scalar.activation(jx[:], wz[:], EXP)

            # rest of the input DMAs (overlap the first lift/score groups)
            dma_gt(1)
            dma_gt(2)
            dma_gt(3)
            for pr in range(1, PAIRS):
                dma_wt(pr)

            # sums[:, (2*pr+h)*16 + qi] = sum_j exp(scores) of that row tile
            sums1 = sums.tile([128, 128], f32, name="sums1", tag="sums")

            qkts = {}

            def lift_chunk(pr, ft, tcn):
                """qkt[pr] cols [ft*2048 + tcn*512, +512) from W f-tile."""
                off = ring_alloc(1)
                pt = ring[:, off:off + 512]
                w0 = pr * 256 + ft * 128
                for e in range(NE):
                    nc.tensor.matmul(
                        pt,
                        lhsT=wT[:, e * 1024 + w0: e * 1024 + w0 + 128],
                        rhs=gT[:, e * 2048 + tcn * 512:
                               e * 2048 + tcn * 512 + 512],
                        start=(e == 0), stop=(e == NE - 1))
                nc.vector.tensor_copy(
                    qkts[pr][:, ft * 2048 + tcn * 512:
                             ft * 2048 + tcn * 512 + 512], pt)

            def score_qtile(pr, qi):
                """Causal scores + exp/rowsum for both heads of pair pr,
                query rows [qi*128, +128). Head matmuls interleaved so they
                run concurrently on disjoint PE row-groups."""
                kneed = 128 * (qi + 1)
                nb = (kneed + 511) // 512
                qkt = qkts[pr]
                offs = [ring_alloc(nb), ring_alloc(nb)]
                lhsTs = [qkt[64 * h:64 * h + 64, qi * 128: qi * 128 + 128]
                         for h in range(2)]
                a = 0
                while a < kneed:
                    sz = min(512, kneed - a)
                    last = (a + sz == kneed)
                    for h in range(2):
                        nc.tensor.matmul(
                            ring[:, offs[h] + a: offs[h] + a + sz],
                            lhsT=lhsTs[h],
                            rhs=qkt[64 * h:64 * h + 64,
                                    2048 + a: 2048 + a + sz],
                            start=True, stop=not last)
                    a += sz
                for h in range(2):
                    # additive causal mask on the diagonal block
                    nc.tensor.matmul(
                        ring[:, offs[h] + kneed - 128: offs[h] + kneed],
                        lhsT=identb[:], rhs=tri[:], start=False, stop=True)
                for h in range(2):
                    col = (2 * pr + h) * 16 + qi
                    reg = ring[:, offs[h]: offs[h] + kneed]
                    if qi < 10:
                        nc.scalar.activation(reg, reg, EXP)
                        nc.vector.reduce_sum(out=sums1[:, col:col + 1],
                                             in_=reg, axis=AX)
                    else:
                        nc.scalar.activation(
                            reg, reg, EXP,
                            accum_out=sums1[:, col:col + 1])

            # ---- schedule -------------------------------------------------
            # Pair 0 lifts itself early (spread over qi0-4); pair p+1's
            # lift rides in pair p's qi window where ring banks are free.
            qkts[0] = qkp.tile([128, 2 * T], bf16, name="qkt0", tag="qkt")
            lift_chunk(0, 0, 0)
            lift_chunk(0, 1, 0)
            p0_self = {0: [(0, 0, 1), (0, 1, 1)], 1: [(0, 0, 2)],
                       2: [(0, 1, 2)], 3: [(0, 0, 3)], 4: [(0, 1, 3)]}
            p0_next = {5: [(1, 0, 0), (1, 1, 0)], 6: [(1, 0, 1)],
                       7: [(1, 1, 1)], 8: [(1, 0, 2)], 9: [(1, 1, 2)],
                       10: [(1, 0, 3)], 11: [(1, 1, 3)]}
            qkts[1] = qkp.tile([128, 2 * T], bf16, name="qkt1", tag="qkt")
            for qi in range(16):
                score_qtile(0, qi)
                for ch in p0_self.get(qi, []):
                    lift_chunk(*ch)
                for ch in p0_next.get(qi, []):
                    lift_chunk(*ch)

            order = [(0, 0), (1, 0), (0, 1), (1, 1),
                     (0, 2), (1, 2), (0, 3), (1, 3)]
            for pr in range(1, PAIRS):
                nxt = pr + 1
                if nxt < PAIRS:
                    qkts[nxt] = qkp.tile([128, 2 * T], bf16,
                                         name=f"qkt{nxt}", tag="qkt")
                for qi in range(16):
                    score_qtile(pr, qi)
                    if nxt < PAIRS and 4 <= qi < 12:
                        ft, tcn = order[qi - 4]
                        lift_chunk(nxt, ft, tcn)

            # ---- finale: lse = ln(sums); out = sum_h w[h]*lse_h -----------
            # s = m * 2^(e-127), m in [1,2):  lse = (e-127)*ln2 + Ln(m)
            u = sums1[:].bitcast(i32)
            ei = fin.tile([128, 128], i32, name="ei", tag="ei")
            nc.vector.tensor_scalar(out=ei[:], in0=u, scalar1=23,
                                    scalar2=None, op0=SHR)
            ef = fin.tile([128, 128], f32, name="ef", tag="ef")
            nc.vector.tensor_copy(ef[:], ei[:])
            nc.vector.tensor_scalar(out=ef[:], in0=ef[:], scalar1=127.0,
                                    scalar2=None, op0=SUB)
            mb = fin.tile([128, 128], i32, name="mb", tag="mb")
            nc.vector.tensor_scalar(out=mb[:], in0=u,
                                    scalar1=0x007FFFFF,
                                    scalar2=0x3F800000,
                                    op0=AND, op1=OR)
            lnm = fin.tile([128, 128], f32, name="lnm", tag="lnm")
            nc.scalar.activation(lnm[:], mb[:].bitcast(f32), LN)
            lse = fin.tile([128, 128], f32, name="lse", tag="lse")
            nc.vector.scalar_tensor_tensor(
                out=lse[:], in0=ef[:], scalar=LN2, in1=lnm[:],
                op0=MUL, op1=ADD)
            wl = fin.tile([128, 128], f32, name="wl", tag="wl")
            nc.vector.tensor_mul(wl[:], lse[:], wrep[:])
            h64 = fin.tile([128, 64], f32, name="h64", tag="h64")
            nc.vector.tensor_add(h64[:], wl[:, 0:64], wl[:, 64:128])
            h32 = fin.tile([128, 32], f32, name="h32", tag="h32")
            nc.vector.tensor_add(h32[:], h64[:, 0:32], h64[:, 32:64])
            facc = fin.tile([128, 16], f32, name="facc", tag="facc")
            nc.vector.tensor_add(facc[:], h32[:, 0:16], h32[:, 16:32])
            nc.sync.dma_start(out=out_d[:], in_=facc[:])

    nc.compile()
    return nc


def _get_nc():
    if "nc" not in _CACHE:
        _CACHE["nc"] = _build()
    return _CACHE["nc"]


def kernel(g, W_lift, W_proj):
    from concourse.bass_utils import run_bass_kernel_spmd

    bf16 = ml_dtypes.bfloat16
    g = np.asarray(g, dtype=np.float32)
    W_lift = np.asarray(W_lift, dtype=np.float32)
    W_proj = np.asarray(W_proj, dtype=np.float32)

    nc = _get_nc()
    w = W_proj.sum(axis=0).astype(np.float32)          # w[h] = sum_g W_proj[g,h]
    tri = np.triu(np.full((128, 128), MASKV, np.float32), k=1).astype(bf16)
    identb = np.eye(128, dtype=np.float32).astype(bf16)

    in_maps = []
    for core in range(NCORES):
        b, hg = core // 2, core % 2
        rows = []
        for p in range(PAIRS):
            h0 = hg * 8 + 2 * p
            h1 = h0 + 1
            rows += list(range(h0 * D, h0 * D + D))
            rows += list(range(h1 * D, h1 * D + D))
            rows += list(range(E + h0 * D, E + h0 * D + D))
            rows += list(range(E + h1 * D, E + h1 * D + D))
        W_slice = W_lift[rows, :]                      # [1024 f, 1024 e]
        wt_host = np.ascontiguousarray(
            W_slice.T.reshape(NE, 128, PAIRS, 256)
            .transpose(2, 0, 1, 3).reshape(4 * E, 256)).astype(bf16)
        gt_host = np.ascontiguousarray(
            g[b].reshape(2, 1024, NE, 128)
            .transpose(0, 2, 3, 1).reshape(T, E)).astype(bf16)
        wrep = np.ascontiguousarray(
            np.broadcast_to(np.repeat(w[hg * 8: hg * 8 + 8], 16)[None, :],
                            (128, 128))).astype(np.float32)
        in_maps.append({
            "gt": gt_host,
            "wt": wt_host,
            "tri": tri,
            "identb": identb,
            "wrep": wrep,
        })

    res = run_bass_kernel_spmd(nc, in_maps, core_ids=list(range(NCORES)))
    _CACHE["last_results"] = res
    _CACHE["last_in_maps"] = in_maps

    out = np.zeros((B, T), dtype=np.float32)
    for core in range(NCORES):
        b = core // 2
        part = res.results[core]["out_part"]           # (128, 16)
        out[b] += part.T.reshape(-1)
    return out
